# revision 5
# baseline (speedup 1.0000x reference)
"""ConcatRelationModule Bass kernel for 8 trn2 NeuronCores — v4.

Per edge e in [0, 16383):
    x      = concat(inputs[heads[e], 0, :], inputs[e + 1, 1, :])     # [512]
    h      = tanh(concat(x @ W_FOH, x @ W_FOM) + b1)                 # [1024]
    h2     = tanh(h @ W2 + b2)                                       # [256]
    out[e] = h2 @ W3 + b3                                            # [E, 64]

v4 (per core, 2048 edges in 4 groups of 512):
  - DMA op count minimized (~16) to stay inside the semaphore pool and
    avoid recycle stalls: two merged constant loads, 5 indirect gathers
    (g0 split in two for an earlier start), 5 XBAR transposes, 4 stores
  - group-interleaved L1(bwd-half -> head-half) -> L2 -> L3 so the PE
    always has non-gather work queued while the serial SWDGE gather
    (~1.4us per 128 rows, gpsimd-only) trickles in
  - modifier half host-pretransposed inside the mega loads
  - PE warm-up matmuls during the prologue (HAM clock gate)
  - b3 added on host; h1/h2 tiles split so L2/L3 start on half-ready data
"""

import os

import numpy as np
import ml_dtypes

import concourse.bass as bass
import concourse.bacc as bacc
import concourse.mybir as mybir
import concourse.tile as tile
from concourse.bass import IndirectOffsetOnAxis
from concourse.bass_utils import run_bass_kernel_spmd

N_TOKENS = 16384
LD = 256
HID = 512
HID2 = 256
NREL = 64
NCORES = 8
E = N_TOKENS - 1
EPC = N_TOKENS // NCORES  # 2048
P = 128
GS = 512
NG = EPC // GS            # 4
SUB = EPC // P            # 16
N_WARMUP = 8

# megaA: w1a [128, 2, 1024] | bwdT groups 0,1 [128, 2, 1024]
MA_W1A = 0
MA_BWD = 2048
MA_COLS = 4096
# megaB: bwdT groups 2,3 [128, 2, 1024] | w1b | w2 [128,8,256] | w3 [128,2,64]
#        | b1 [128,8] | b2 [128,2]  (all bf16)
MB_BWD = 0
MB_W1B = 2048
MB_W2 = 4096
MB_W3 = 6144
MB_B1 = 6272
MB_B2 = 6280
MB_COLS = 6282

LAST_RESULTS = None
_CACHE = {}


def _build():
    bf16 = mybir.dt.bfloat16
    f32 = mybir.dt.float32

    nc = bacc.Bacc()
    fwd = nc.declare_dram_parameter("fwd", [N_TOKENS, LD], bf16, isOutput=False)
    headsT = nc.declare_dram_parameter(
        "headsT", [P, SUB], mybir.dt.int32, isOutput=False)
    megaA = nc.declare_dram_parameter("megaA", [P, MA_COLS], bf16, isOutput=False)
    megaB = nc.declare_dram_parameter("megaB", [P, MB_COLS], bf16, isOutput=False)
    outT = nc.declare_dram_parameter("outT", [NREL, EPC], f32, isOutput=True)

    Tanh = mybir.ActivationFunctionType.Tanh

    with tile.TileContext(nc) as tc:
        with (
            tc.tile_pool(name="const", bufs=1) as const_pool,
            tc.tile_pool(name="xh", bufs=NG) as xh_pool,
            tc.tile_pool(name="xT", bufs=NG) as xT_pool,
            tc.tile_pool(name="h1", bufs=2 * NG) as h1_pool,
            tc.tile_pool(name="h2", bufs=2 * NG) as h2_pool,
            tc.tile_pool(name="o", bufs=4) as o_pool,
            tc.tile_pool(name="ps", bufs=8, space="PSUM") as ps_pool,
        ):
            # headsT first on the (hot) Sync queue
            hT_sb = const_pool.tile([P, SUB], mybir.dt.int32)
            nc.sync.dma_start(hT_sb[:], headsT[:])
            mA = const_pool.tile([P, MA_COLS], bf16)
            nc.sync.dma_start(mA[:], megaA[:])
            mB = const_pool.tile([P, MB_COLS], bf16)
            nc.sync.dma_start(mB[:], megaB[:])

            def w1a(i, hc):  # modifier-half W1 chunk [128, 128]
                c = MA_W1A + i * 1024 + hc * P
                return mA[:, c:c + P]

            def w1b(i, hc):  # head-half W1 chunk
                c = MB_W1B + i * 1024 + hc * P
                return mB[:, c:c + P]

            def bwd_rhs(i, g):  # modifier features i*128.. for group g
                if g < 2:
                    c = MA_BWD + i * 1024 + g * GS
                    return mA[:, c:c + GS]
                c = MB_BWD + i * 1024 + (g - 2) * GS
                return mB[:, c:c + GS]

            def w2c(kc, jc):
                c = MB_W2 + kc * HID2 + jc * P
                return mB[:, c:c + P]

            def w3c(kc):
                c = MB_W3 + kc * NREL
                return mB[:, c:c + NREL]

            def bias1(hc):
                return mB[:, MB_B1 + hc:MB_B1 + hc + 1]

            def bias2(jc):
                return mB[:, MB_B2 + jc:MB_B2 + jc + 1]

            # gathers (gpsimd SWDGE): group 0 as two 2-subtile gathers for an
            # earlier first transpose; groups 1-3 as one 4-subtile gather each
            warm_sb = const_pool.tile([P, GS], bf16)
            nc.gpsimd.memset(warm_sb[:], 0)
            xh = []
            for g in range(NG):
                t = xh_pool.tile([P, 4, LD], bf16, tag="xh", name=f"xh_{g}")
                for sl in range(4):
                    nc.gpsimd.indirect_dma_start(
                        out=t[:, sl, :],
                        out_offset=None,
                        in_=fwd[:],
                        in_offset=IndirectOffsetOnAxis(
                            ap=hT_sb[:, g * 4 + sl:g * 4 + sl + 1], axis=0),
                    )
                xh.append(t)

            # PE warm-up on scratch; output never read
            wps = ps_pool.tile([P, GS], f32, tag="ps", name="warmup")
            for i in range(N_WARMUP):
                nc.tensor.matmul(
                    out=wps[:], lhsT=warm_sb[:, 0:P], rhs=warm_sb[:],
                    start=True, stop=True,
                )

            # XBAR transpose per group: xT[p, m, e] = xh[e, m*128+p], m=sl*2+kc
            xT = []

            def emit_transpose(g):
                tg = xT_pool.tile([P, 8, P], bf16, tag="xT", name=f"xT_{g}")
                if g == 0:
                    for half in range(2):
                        nc.sync.dma_start_transpose(
                            out=tg[:, 4 * half:4 * half + 4, :],
                            in_=xh[g][:, 2 * half:2 * half + 2, :])
                else:
                    nc.sync.dma_start_transpose(out=tg[:], in_=xh[g][:])
                xT.append(tg)

            def head_rhs(g, kc):  # [128, 4, 128] strided view, m = sl*2 + kc
                return xT[g][:].rearrange("p (s k) e -> p k s e", k=2)[:, kc, :, :]

            # ---- main loop: per group L1 (bwd first) -> L2 -> L3 ----
            for g in range(NG):
                emit_transpose(g)

                ph = [ps_pool.tile([P, GS], f32, tag="ps", name=f"ph_{g}_{hc}")
                      for hc in range(8)]
                for hc in range(8):  # modifier half first: no gather dep
                    for i in range(2):
                        nc.tensor.matmul(
                            out=ph[hc][:], lhsT=w1a(i, hc), rhs=bwd_rhs(i, g),
                            start=(i == 0), stop=False,
                        )
                h1g = [h1_pool.tile([P, 4, GS], bf16, tag="h1",
                                    name=f"h1_{g}_{half}") for half in range(2)]
                for hc in range(8):
                    for i in range(2):
                        nc.tensor.matmul(
                            out=ph[hc][:], lhsT=w1b(i, hc), rhs=head_rhs(g, i),
                            start=False, stop=(i == 1),
                        )
                    nc.scalar.activation(
                        out=h1g[hc // 4][:, hc % 4, :], in_=ph[hc][:],
                        func=Tanh, bias=bias1(hc),
                    )

                h2g = [h2_pool.tile([P, GS], bf16, tag="h2",
                                    name=f"h2_{g}_{jc}") for jc in range(2)]
                for jc in range(2):
                    pj = ps_pool.tile([P, GS], f32, tag="ps", name=f"pj_{g}_{jc}")
                    for kc in range(8):
                        nc.tensor.matmul(
                            out=pj[:], lhsT=w2c(kc, jc),
                            rhs=h1g[kc // 4][:, kc % 4, :],
                            start=(kc == 0), stop=(kc == 7),
                        )
                    nc.scalar.activation(
                        out=h2g[jc][:], in_=pj[:], func=Tanh, bias=bias2(jc),
                    )

                po = ps_pool.tile([NREL, GS], f32, tag="ps", name=f"po_{g}")
                for kc in range(2):
                    nc.tensor.matmul(
                        out=po[:], lhsT=w3c(kc), rhs=h2g[kc][:],
                        start=(kc == 0), stop=(kc == 1),
                    )
                og = o_pool.tile([NREL, GS], f32, tag="o", name=f"o_{g}")
                nc.vector.tensor_copy(out=og[:], in_=po[:])
                # stores on Scalar: keeps the Sync queue free for the group
                # transposes (no head-of-line blocking on the copy sem)
                nc.scalar.dma_start(outT[:, g * GS:(g + 1) * GS], og[:])

    nc.finalize()
    return nc


def _prep_inputs(inputs, rhidLayerFOH, rhidLayerFOM, rcatBias, rhid2Layer,
                 rhid2Bias, routLayer, routBias, heads):
    wdt = ml_dtypes.bfloat16
    inputs = np.asarray(inputs, dtype=np.float32)
    heads = np.asarray(heads)

    fwd = np.ascontiguousarray(inputs[:, 0, :]).astype(wdt)
    bwd_full = inputs[:, 1, :]
    mods_pad = np.concatenate(
        [np.arange(1, N_TOKENS), [N_TOKENS - 1]]).astype(np.int64)
    heads_pad = np.concatenate([heads.astype(np.int64), [0]]).astype(np.int32)

    w1 = np.concatenate(
        [np.asarray(rhidLayerFOH), np.asarray(rhidLayerFOM)],
        axis=1).astype(np.float32)                               # [512, 1024]
    # [128, 2, 1024]: chunk i, partition p -> w1 row base + i*128 + p
    w1a = w1[2 * P:].reshape(2, P, 2 * HID).transpose(1, 0, 2)   # modifier rows
    w1b = w1[:2 * P].reshape(2, P, 2 * HID).transpose(1, 0, 2)   # head rows
    w2p = (np.asarray(rhid2Layer, dtype=np.float32)
           .reshape(8, P, HID2).transpose(1, 0, 2))              # [128, 8, 256]
    w3p = (np.asarray(routLayer, dtype=np.float32)
           .reshape(2, P, NREL).transpose(1, 0, 2))              # [128, 2, 64]
    b1 = np.asarray(rcatBias, dtype=np.float32).reshape(8, P).T  # [128, 8]
    b2 = np.asarray(rhid2Bias, dtype=np.float32).reshape(2, P).T  # [128, 2]

    in_maps = []
    for c in range(NCORES):
        sl = slice(c * EPC, (c + 1) * EPC)
        bwd_c = bwd_full[mods_pad[sl]]                           # [2048, 256]
        bwdT_c = bwd_c.T.reshape(2, P, EPC).transpose(1, 0, 2)   # [128, 2, 2048]
        megaA = np.concatenate(
            [w1a.reshape(P, -1), bwdT_c[:, :, :2 * GS].reshape(P, -1)],
            axis=1)                                              # [128, 4096]
        megaB = np.concatenate(
            [bwdT_c[:, :, 2 * GS:].reshape(P, -1), w1b.reshape(P, -1),
             w2p.reshape(P, -1), w3p.reshape(P, -1), b1, b2],
            axis=1)                                              # [128, 6282]
        headsT_c = np.ascontiguousarray(heads_pad[sl].reshape(SUB, P).T)
        in_maps.append({
            "fwd": fwd,
            "headsT": headsT_c,
            "megaA": np.ascontiguousarray(megaA).astype(wdt),
            "megaB": np.ascontiguousarray(megaB).astype(wdt),
        })
    return in_maps


def kernel(inputs, rhidLayerFOH, rhidLayerFOM, rcatBias, rhid2Layer, rhid2Bias,
           routLayer, routBias, heads):
    global LAST_RESULTS

    in_maps = _prep_inputs(inputs, rhidLayerFOH, rhidLayerFOM, rcatBias,
                           rhid2Layer, rhid2Bias, routLayer, routBias, heads)

    if "nc" not in _CACHE:
        _CACHE["nc"] = _build()
    nc = _CACHE["nc"]

    trace_dir = os.environ.get("KERNEL_TRACE_DIR") or None
    res = run_bass_kernel_spmd(nc, in_maps, list(range(NCORES)), tmpdir=trace_dir)
    LAST_RESULTS = res

    outT = np.concatenate([r["outT"] for r in res.results], axis=1)
    out = outT.T[:E] + np.asarray(routBias, dtype=np.float32)[None, :]
    return np.ascontiguousarray(out).astype(np.float32)
